# revision 7
# baseline (speedup 1.0000x reference)
"""Masked dot-product attention (B=64, Lq=Lk=1024, d=64, fp32) on 8 TRN2 cores.

v3 strategy (batch-parallel, 8 batch slots per core):
  - S-matmul in fp8e4 DoubleRow mode at 0.5 cycles/row: the 256-row
    contraction budget (128 partitions x 2) packs the FULL residual
    expansion  S = Qhi*Khi + Qhi*Klo + Qlo*Khi + Qlo*Klo  where
    Qhi = e4m3(Q/sqrt8), Qlo = e4m3(Q/sqrt8 - Qhi) (same for K), giving
    ~fp16 score accuracy at HALF the fp16 matmul cost (PE cost model is
    out-free-size * cycles_per_row, contraction depth is free).
  - Masking without a mask row: host zeroes masked K columns (scores
    become 0 -> exp = 1) AND masked V rows incl. the denominator ones-
    column, so masked keys contribute exactly 0 to numerator and
    denominator.
  - exp is 2-way split across ACT (exact exp, 1038ns/tile) and DVE
    (Schraudolph int16 fast-exp, 1192ns/tile) with a rate-balanced
    greedy tile assignment; these two engines are the only PSUM readers
    and are the kernel bottleneck (~23us of elementwise work).
  - O accumulates per q-subtile in fp16: out[128q, 65] += P^T @ [V|1],
    cumulative across batch slots (no bank-zeroing matmuls; host
    recovers per-batch values by differencing fp32 snapshots).  The
    last (smallest) slot zero-opens its banks so its result ships
    exact fp16, shortening the end-of-kernel DMA drain.
  - Per-slot PSUM snapshots [128,2,260] are copied bank-per-engine
    (ACT || DVE) and DMA'd on alternating queues.
  - Raggedness: batches sorted by active k-tile count and dealt across
    cores (slot s runs max-of-group tiles on every core); slot order is
    big->small so the tail chain is the 1-tile batch.
  - Software-pipelined emission: S of tile i+LOOKAHEAD precedes O of
    tile i; PE (16us busy) never gates the exp engines.
"""

import numpy as np
import ml_dtypes

import concourse.bass as bass
import concourse.mybir as mybir
import concourse.tile as tile
from concourse import bacc
from concourse.bass_utils import run_bass_kernel_spmd

N_CORES = 8
B = 64
L = 1024
D = 64
BPC = B // N_CORES   # batch slots per core
KT = L // 128        # max k-tiles per batch

F16 = mybir.dt.float16
F32 = mybir.dt.float32
F8 = mybir.dt.float8e4
I16 = mybir.dt.int16
E4 = ml_dtypes.float8_e4m3

# Schraudolph fp16 fast-exp constants (C=44.75 minimax-tuned: 3.0% max err)
EXP_A = 1477.3197
EXP_B = 15315.25

# engine cost model (ns) for balancing the ACT/DVE split
ACT_TILE = 1038.0
DVE_TILE = 1192.0
ACT_COPY = 370.0
DVE_COPY = 396.0

VPW = 260 + (KT - 1) * (D + 1)

_prog_cache = {}


def _exp_assignment(n_jobs, n_copies):
    """Greedy rate-balanced assignment. Returns act_tile[g] bool list and
    per-copy engine hints (copies interleave at batch ends; approximate by
    spreading their cost evenly)."""
    # assign tiles one by one to the engine that would finish it earlier,
    # pre-charging each engine half the total copy work
    act_load = n_copies * ACT_COPY
    dve_load = n_copies * DVE_COPY
    act_tile = []
    for g in range(n_jobs):
        if act_load + ACT_TILE <= dve_load + DVE_TILE:
            act_tile.append(True)
            act_load += ACT_TILE
        else:
            act_tile.append(False)
            dve_load += DVE_TILE
    return act_tile


def _build_program(ns):
    """ns: per-slot k-tile counts (tuple of BPC ints in 1..KT), big->small."""
    nc = bacc.Bacc("TRN2", target_bir_lowering=False, debug=False,
                   num_devices=N_CORES)
    # Q-side fp8 4-term pack: [slot][128, 2, 1024]
    qop_d = nc.dram_tensor("qop", [BPC, 128, 2, L], F8, kind="ExternalInput")
    # K-side fp8 4-term pack per k-tile: [slot][128, kt, 2, 128]
    kop_d = nc.dram_tensor("kop", [BPC, 128, KT, 2, 128], F8,
                           kind="ExternalInput")
    # [V|1] fp16, tile0 zero-padded to 260 for the bank-opening matmuls
    vp_d = nc.dram_tensor("vp", [BPC, 128, VPW], F16, kind="ExternalInput")
    # fp32 cumulative accumulator snapshots for slots 0..BPC-2
    o_d = nc.dram_tensor("o", [BPC - 1, 128, 2, 4 * (D + 1)], F32,
                         kind="ExternalOutput")
    # last slot zero-opens its banks -> exact fp16
    o16_d = nc.dram_tensor("o16", [128, 2, 4 * (D + 1)], F16,
                           kind="ExternalOutput")

    jobs = [(b, kt) for b in range(BPC) for kt in range(ns[b])]
    n_jobs = len(jobs)
    act_tile = _exp_assignment(n_jobs, BPC)

    with tile.TileContext(nc) as tc:
        with (
            tc.tile_pool(name="qop", bufs=2) as qop_pool,
            tc.tile_pool(name="kop", bufs=2) as kop_pool,
            tc.tile_pool(name="vpp", bufs=2) as vp_pool,
            tc.tile_pool(name="pt", bufs=7) as pt_pool,
            tc.tile_pool(name="osb", bufs=3) as osb_pool,
            tc.tile_pool(name="sp", bufs=3, space="PSUM") as sp_pool,
            tc.tile_pool(name="op", bufs=1, space="PSUM") as op_pool,
        ):
            qop_s = [None] * BPC
            kop_s = [None] * BPC
            vp_s = [None] * BPC
            opsum = [None] * BPC
            pt_of_job = [None] * n_jobs
            started = [False] * (BPC + 1)

            # warm the ACT exp table (1.3us load) during the DMA-latency head
            sc = osb_pool.tile([128, 1], F32, tag="sc", name="sc")
            nc.gpsimd.memset(sc[:], 0.0)
            nc.scalar.activation(sc[:], sc[:],
                                 mybir.ActivationFunctionType.Exp)

            # PE p-state warmup: ~24 throwaway fp16 matmuls on a zeroed dummy
            # keep PE continuously busy through the ~3us DMA-latency head, so
            # the real matmuls run at the full 2.4GHz clock from the start.
            # They write (start=True, stop=True) into the op banks, which are
            # re-opened by slot 0 / the last slot later.
            dummy = osb_pool.tile([128, 256], F16, tag="dummy", name="dummy")
            nc.gpsimd.memset(dummy[:], 0.0)
            warm = [
                op_pool.tile([128, 512], F32, tag=f"oph{h}", name=f"warm{h}")
                for h in range(2)
            ]
            for w in range(24):
                nc.tensor.matmul(warm[w % 2][:, :256], dummy[:, :128],
                                 dummy[:], start=True, stop=True,
                                 skip_group_check=True)

            def start_batch(b):
                nkt = ns[b]
                vend = 260 + (nkt - 1) * (D + 1)
                qop = qop_pool.tile([128, 2, L], F8, tag="qop",
                                    name=f"qop{b}")
                kop = kop_pool.tile([128, KT, 2, 128], F8, tag="kop",
                                    name=f"kop{b}")
                vp = vp_pool.tile([128, VPW], F16, tag="vp", name=f"vp{b}")
                nc.sync.dma_start(kop[:, :nkt], kop_d[b][:, :nkt])
                if b == 0:
                    # lead chunk: the first S-matmul only needs q-cols 0:256
                    nc.sync.dma_start(qop[:, :, :256], qop_d[b][:, :, :256])
                    nc.sync.dma_start(qop[:, :, 256:], qop_d[b][:, :, 256:])
                else:
                    nc.sync.dma_start(qop[:], qop_d[b])
                nc.sync.dma_start(vp[:, :vend], vp_d[b][:, :vend])
                qop_s[b] = qop
                kop_s[b] = kop
                vp_s[b] = vp

            def ensure_started(b):
                if b < BPC and not started[b]:
                    started[b] = True
                    start_batch(b)

            sp_of_job = [None] * n_jobs

            def s_mm(g):
                b, kt = jobs[g]
                ensure_started(b)
                # prefetch inputs for every batch reached within 6 jobs
                tb = jobs[min(g + 6, n_jobs - 1)][0]
                for bb in range(b + 1, tb + 1):
                    ensure_started(bb)
                sp = sp_pool.tile([128, L], F32, tag="sp", name=f"sp{g}")
                pt = pt_pool.tile([128, L], F16, tag="pt", name=f"pt{g}")
                sp_of_job[g] = sp
                pt_of_job[g] = pt
                ktile = kop_s[b][:, kt]   # [128, 2, 128]
                if g == 0:
                    # 256-col chunks: the first chunk only needs the qop lead
                    # DMA, so the first exp starts as early as possible
                    for qo in range(0, L, 256):
                        nc.tensor.matmul(
                            sp[:, qo:qo + 256], ktile,
                            qop_s[b][:, :, qo:qo + 256],
                            start=True, stop=True,
                            perf_mode=mybir.MatmulPerfMode.DoubleRow)
                else:
                    for qo in range(0, L, 512):
                        nc.tensor.matmul(
                            sp[:, qo:qo + 512], ktile,
                            qop_s[b][:, :, qo:qo + 512],
                            start=True, stop=True,
                            perf_mode=mybir.MatmulPerfMode.DoubleRow)

            def exp_emit(g):
                sp = sp_of_job[g]
                pt = pt_of_job[g]
                sp_of_job[g] = None
                if act_tile[g]:
                    nc.scalar.activation(pt[:], sp[:],
                                         mybir.ActivationFunctionType.Exp)
                else:
                    nc.vector.tensor_scalar(
                        pt.bitcast(I16)[:], sp[:], EXP_A, EXP_B,
                        mybir.AluOpType.mult, mybir.AluOpType.add)

            def o_mm(g):
                b, kt = jobs[g]
                nkt = ns[b]
                if kt == 0:
                    opsum[b] = [
                        op_pool.tile([128, 512], F32, tag=f"oph{h}",
                                     name=f"op{b}h{h}")
                        for h in range(2)
                    ]
                pt = pt_of_job[g]
                last = kt == nkt - 1
                if kt == 0 and (b == 0 or b == BPC - 1):
                    # open each bank with a zero-padded matmul: slot 0 opens
                    # undefined PSUM; the last slot opts out of the stale-
                    # accumulate chain so its output ships exact fp16
                    vz = vp_s[b][:, :260]  # [V|1|zeros(195)]
                    for h in range(2):
                        nc.tensor.matmul(
                            opsum[b][h][:, :260],
                            pt[:, h * 512:h * 512 + 128], vz,
                            start=True, stop=False,
                            skip_group_check=True)
                    qlist = [1, 2, 3, 5, 6, 7]
                else:
                    qlist = list(range(8))
                vt = vp_s[b][:, kt * (D + 1) + 195:kt * (D + 1) + 260] \
                    if kt > 0 else vp_s[b][:, :D + 1]
                for q in qlist:
                    h, j = divmod(q, 4)
                    nc.tensor.matmul(
                        opsum[b][h][:, j * (D + 1):(j + 1) * (D + 1)],
                        pt[:, q * 128:(q + 1) * 128],
                        vt,
                        start=False,
                        stop=last and (j == 3),
                        skip_group_check=True,
                    )
                pt_of_job[g] = None
                if last and b == BPC - 1:
                    # tail: per-bank copies in parallel (ACT || DVE), fp16,
                    # each bank DMA'd as soon as its copy lands (SP queue is
                    # idle at the end; HWDGE latency beats Pool SWDGE)
                    osbL = osb_pool.tile([128, 2, 4 * (D + 1)], F16,
                                         tag="osbL", name="osbL")
                    nc.scalar.copy(osbL[:, 0, :], opsum[b][0][:, :4 * (D + 1)])
                    nc.sync.dma_start(o16_d[:, 0], osbL[:, 0, :])
                    nc.vector.tensor_copy(osbL[:, 1, :],
                                          opsum[b][1][:, :4 * (D + 1)])
                    nc.sync.dma_start(o16_d[:, 1], osbL[:, 1, :])
                elif last:
                    osb = osb_pool.tile([128, 2, 4 * (D + 1)], F32,
                                        tag="osb", name=f"osb{b}")
                    nc.scalar.copy(osb[:, 0, :], opsum[b][0][:, :4 * (D + 1)])
                    nc.vector.tensor_copy(osb[:, 1, :],
                                          opsum[b][1][:, :4 * (D + 1)])
                    # Pool SWDGE: keeps the in-order SP queue free for input
                    # prefetch (a blocked output DMA would stall it)
                    nc.gpsimd.dma_start(o_d[b], osb[:])

            LOOKAHEAD = 5
            for g in range(n_jobs):
                s_mm(g)
                if g >= LOOKAHEAD:
                    o_mm(g - LOOKAHEAD)
                exp_emit(g)
            for g in range(max(0, n_jobs - LOOKAHEAD), n_jobs):
                o_mm(g)

    nc.compile()
    return nc


def get_program(ns):
    ns = tuple(ns)
    if ns not in _prog_cache:
        _prog_cache[ns] = _build_program(ns)
    return _prog_cache[ns]


def _prep_inputs(q, k, v, vl):
    """q,k,v: [n, L, D] fp32; vl: [n] int (vl>0).
    Returns (qop fp8 [n,128,2,L], kop fp8 [n,128,KT,2,128],
             vp fp16 [n,128,VPW])."""
    n = q.shape[0]
    s = np.float32(1.0 / np.sqrt(8.0))
    qt = (q.transpose(0, 2, 1) * s).astype(np.float32)   # [n, D, L]
    kt_ = (k.transpose(0, 2, 1) * s).astype(np.float32)  # [n, D, L]
    iota = np.arange(L)
    kmask = (iota[None, :] < vl[:, None])                # [n, L] valid keys
    kt_ = kt_ * kmask[:, None, :]

    qhi8 = qt.astype(E4)
    qlo8 = (qt - qhi8.astype(np.float32)).astype(E4)
    khi8 = kt_.astype(E4)
    klo8 = (kt_ - khi8.astype(np.float32)).astype(E4)

    # 4-term pack: partitions p<64 -> Qhi[d], p>=64 -> Qlo[d]; dim1 r is the
    # DoubleRow pair index; K side: r=0 -> Khi, r=1 -> Klo for p<64 and
    # r=0 -> Khi, r=1 -> Klo for p>=64 as well, so that the (p, r) terms are
    # (QhiKhi, QhiKlo, QloKhi, QloKlo).
    qop = np.empty((n, 128, 2, L), E4)
    qop[:, :D, 0] = qhi8
    qop[:, :D, 1] = qhi8
    qop[:, D:, 0] = qlo8
    qop[:, D:, 1] = qlo8

    kop = np.empty((n, 128, KT, 2, 128), E4)
    kk_hi = khi8.reshape(n, D, KT, 128)
    kk_lo = klo8.reshape(n, D, KT, 128)
    kop[:, :D, :, 0] = kk_hi
    kop[:, :D, :, 1] = kk_lo
    kop[:, D:, :, 0] = kk_hi
    kop[:, D:, :, 1] = kk_lo

    vpe = np.empty((n, L, D + 1), np.float16)
    vpe[:, :, :D] = v.astype(np.float16)
    vpe[:, :, D] = 1.0
    vpe *= kmask[:, :, None]     # masked keys contribute 0 to num and den
    vpe = vpe.reshape(n, KT, 128, D + 1).transpose(0, 2, 1, 3)
    vp = np.zeros((n, 128, VPW), np.float16)
    vp[:, :, :D + 1] = vpe[:, :, 0, :]
    vp[:, :, 260:] = vpe[:, :, 1:, :].reshape(n, 128, (KT - 1) * (D + 1))
    return qop, kop, vp


def kernel(queries, keys, values, valid_lens):
    queries = np.asarray(queries, np.float32)
    keys = np.asarray(keys, np.float32)
    values = np.asarray(values, np.float32)
    vl = np.asarray(valid_lens).astype(np.int64)

    # vl==0 -> reference softmaxes constant NEG_INF -> uniform over ALL keys
    zmask = vl == 0
    vl_dev = np.where(zmask, L, vl)

    # Ragged load balancing: sort batches by active k-tile count, deal
    # across cores (slot s <- sorted group). Slot order big->small: a big
    # first batch covers the early input-DMA ramp; the smallest last batch
    # leaves the shortest drain chain.
    nact = (-(-vl_dev // 128)).astype(np.int64)
    order = np.argsort(nact, kind="stable")
    # groups sorted ascending by size; slot order interleaves big and small
    # so snapshot copies spread through the job stream, with the biggest
    # first (covers the input-DMA ramp) and the smallest last (short drain)
    slot_groups = [7, 2, 5, 3, 6, 4, 1, 0]
    ns = tuple(int(nact[order[g * N_CORES + N_CORES - 1]])
               for g in slot_groups)

    qop, kop, vp = _prep_inputs(queries[order], keys[order], values[order],
                                vl_dev[order])

    nc = get_program(ns)
    in_maps = []
    for c in range(N_CORES):
        idx = [slot_groups[s] * N_CORES + c for s in range(BPC)]
        in_maps.append({
            "qop": np.ascontiguousarray(qop[idx]),
            "kop": np.ascontiguousarray(kop[idx]),
            "vp": np.ascontiguousarray(vp[idx]),
        })

    res = None
    for attempt in range(3):
        try:
            res = run_bass_kernel_spmd(nc, in_maps, list(range(N_CORES)))
            break
        except Exception:
            # Transient NRT/axon device failures have been observed on the
            # first execution of a freshly compiled NEFF; reset and retry.
            if attempt == 2:
                raise
            import time as _time
            _time.sleep(2.0)
            try:
                import jax
                jax.clear_caches()
            except Exception:
                pass

    out = np.empty((B, L, D), np.float32)
    for c in range(N_CORES):
        raw = res.results[c]["o"]  # [BPC-1,128,2,260] cumulative
        o = np.concatenate([
            raw[:1], np.diff(raw, axis=0),
            res.results[c]["o16"][None].astype(np.float32),
        ])
        o = o.reshape(BPC, 128, 2, 4, D + 1).transpose(0, 2, 3, 1, 4)
        o = o.reshape(BPC, L, D + 1)  # rows [(4h+j)*128 + p]
        on = o[:, :, :D] / o[:, :, D:D + 1]
        for s in range(BPC):
            out[order[slot_groups[s] * N_CORES + c]] = on[s]

    if zmask.any():
        out[zmask] = values[zmask].mean(axis=1, keepdims=True)
    return out


# revision 8
# speedup vs baseline: 1.0433x; 1.0433x over previous
"""Masked dot-product attention (B=64, Lq=Lk=1024, d=64, fp32) on 8 TRN2 cores.

v3 strategy (batch-parallel, 8 batch slots per core):
  - S-matmul in fp8e4 DoubleRow mode at 0.5 cycles/row: the 256-row
    contraction budget (128 partitions x 2) packs the FULL residual
    expansion  S = Qhi*Khi + Qhi*Klo + Qlo*Khi + Qlo*Klo  where
    Qhi = e4m3(Q/sqrt8), Qlo = e4m3(Q/sqrt8 - Qhi) (same for K), giving
    ~fp16 score accuracy at HALF the fp16 matmul cost (PE cost model is
    out-free-size * cycles_per_row, contraction depth is free).
  - Masking without a mask row: host zeroes masked K columns (scores
    become 0 -> exp = 1) AND masked V rows incl. the denominator ones-
    column, so masked keys contribute exactly 0 to numerator and
    denominator.
  - exp is 2-way split across ACT (exact exp, 1038ns/tile) and DVE
    (Schraudolph int16 fast-exp, 1192ns/tile) with a rate-balanced
    greedy tile assignment; these two engines are the only PSUM readers
    and are the kernel bottleneck (~23us of elementwise work).
  - O accumulates per q-subtile in fp16: out[128q, 65] += P^T @ [V|1],
    cumulative across batch slots (no bank-zeroing matmuls; host
    recovers per-batch values by differencing fp32 snapshots).  The
    last (smallest) slot zero-opens its banks so its result ships
    exact fp16, shortening the end-of-kernel DMA drain.
  - Per-slot PSUM snapshots [128,2,260] are copied bank-per-engine
    (ACT || DVE) and DMA'd on alternating queues.
  - Raggedness: batches sorted by active k-tile count and dealt across
    cores (slot s runs max-of-group tiles on every core); slot order is
    big->small so the tail chain is the 1-tile batch.
  - Software-pipelined emission: S of tile i+LOOKAHEAD precedes O of
    tile i; PE (16us busy) never gates the exp engines.
"""

import numpy as np
import ml_dtypes

import concourse.bass as bass
import concourse.mybir as mybir
import concourse.tile as tile
from concourse import bacc
from concourse.bass_utils import run_bass_kernel_spmd

N_CORES = 8
B = 64
L = 1024
D = 64
BPC = B // N_CORES   # batch slots per core
KT = L // 128        # max k-tiles per batch

F16 = mybir.dt.float16
F32 = mybir.dt.float32
F8 = mybir.dt.float8e4
I16 = mybir.dt.int16
E4 = ml_dtypes.float8_e4m3

# Schraudolph fp16 fast-exp constants (C=44.75 minimax-tuned: 3.0% max err)
EXP_A = 1477.3197
EXP_B = 15315.25

# engine cost model (ns) for balancing the ACT/DVE split
ACT_TILE = 1038.0
DVE_TILE = 1192.0
ACT_COPY = 370.0
DVE_COPY = 396.0

VPW = 260 + (KT - 1) * (D + 1)

_prog_cache = {}


def _exp_assignment(n_jobs, n_copies):
    """Greedy rate-balanced assignment. Returns act_tile[g] bool list and
    per-copy engine hints (copies interleave at batch ends; approximate by
    spreading their cost evenly)."""
    # assign tiles one by one to the engine that would finish it earlier,
    # pre-charging each engine half the total copy work
    act_load = n_copies * ACT_COPY
    dve_load = n_copies * DVE_COPY
    act_tile = []
    for g in range(n_jobs):
        if act_load + ACT_TILE <= dve_load + DVE_TILE:
            act_tile.append(True)
            act_load += ACT_TILE
        else:
            act_tile.append(False)
            dve_load += DVE_TILE
    return act_tile


def _build_program(ns):
    """ns: per-slot k-tile counts (tuple of BPC ints in 1..KT), big->small."""
    nc = bacc.Bacc("TRN2", target_bir_lowering=False, debug=False,
                   num_devices=N_CORES)
    # Q-side fp8 4-term pack: [slot][128, 2, 1024]
    qop_d = nc.dram_tensor("qop", [BPC, 128, 2, L], F8, kind="ExternalInput")
    # K-side fp8 4-term pack per k-tile: [slot][128, kt, 2, 128]
    kop_d = nc.dram_tensor("kop", [BPC, 128, KT, 2, 128], F8,
                           kind="ExternalInput")
    # [V|1] fp16, tile0 zero-padded to 260 for the bank-opening matmuls
    vp_d = nc.dram_tensor("vp", [BPC, 128, VPW], F16, kind="ExternalInput")
    # fp32 cumulative accumulator snapshots for slots 0..BPC-2
    o_d = nc.dram_tensor("o", [BPC - 1, 128, 2, 4 * (D + 1)], F32,
                         kind="ExternalOutput")
    # last slot zero-opens its banks -> exact fp16
    o16_d = nc.dram_tensor("o16", [128, 2, 4 * (D + 1)], F16,
                           kind="ExternalOutput")

    jobs = [(b, kt) for b in range(BPC) for kt in range(ns[b])]
    n_jobs = len(jobs)
    act_tile = _exp_assignment(n_jobs, BPC)

    with tile.TileContext(nc) as tc:
        with (
            tc.tile_pool(name="qop", bufs=2) as qop_pool,
            tc.tile_pool(name="kop", bufs=2) as kop_pool,
            tc.tile_pool(name="vpp", bufs=2) as vp_pool,
            tc.tile_pool(name="pt", bufs=7) as pt_pool,
            tc.tile_pool(name="osb", bufs=3) as osb_pool,
            tc.tile_pool(name="sp", bufs=3, space="PSUM") as sp_pool,
            tc.tile_pool(name="op", bufs=1, space="PSUM") as op_pool,
        ):
            qop_s = [None] * BPC
            kop_s = [None] * BPC
            vp_s = [None] * BPC
            opsum = [None] * BPC
            pt_of_job = [None] * n_jobs
            started = [False] * (BPC + 1)

            # warm the ACT exp table (1.3us load) during the DMA-latency head
            sc = osb_pool.tile([128, 1], F32, tag="sc", name="sc")
            nc.gpsimd.memset(sc[:], 0.0)
            nc.scalar.activation(sc[:], sc[:],
                                 mybir.ActivationFunctionType.Exp)

            # PE p-state warmup: a few throwaway matmuls on a zeroed dummy
            # start the clock ramp during the DMA-latency head. Short enough
            # to drain before the first real S-matmul's data lands.
            dummy = osb_pool.tile([128, 256], F16, tag="dummy", name="dummy")
            nc.gpsimd.memset(dummy[:], 0.0)
            warm = [
                op_pool.tile([128, 512], F32, tag=f"oph{h}", name=f"warm{h}")
                for h in range(2)
            ]
            for w in range(6):
                nc.tensor.matmul(warm[w % 2][:, :256], dummy[:, :128],
                                 dummy[:], start=True, stop=True,
                                 skip_group_check=True)

            def start_batch(b):
                nkt = ns[b]
                vend = 260 + (nkt - 1) * (D + 1)
                qop = qop_pool.tile([128, 2, L], F8, tag="qop",
                                    name=f"qop{b}")
                kop = kop_pool.tile([128, KT, 2, 128], F8, tag="kop",
                                    name=f"kop{b}")
                vp = vp_pool.tile([128, VPW], F16, tag="vp", name=f"vp{b}")
                nc.sync.dma_start(kop[:, :nkt], kop_d[b][:, :nkt])
                if b == 0:
                    # lead chunk: the first S-matmul only needs q-cols 0:256
                    nc.sync.dma_start(qop[:, :, :256], qop_d[b][:, :, :256])
                    nc.sync.dma_start(qop[:, :, 256:], qop_d[b][:, :, 256:])
                else:
                    nc.sync.dma_start(qop[:], qop_d[b])
                nc.sync.dma_start(vp[:, :vend], vp_d[b][:, :vend])
                qop_s[b] = qop
                kop_s[b] = kop
                vp_s[b] = vp

            def ensure_started(b):
                if b < BPC and not started[b]:
                    started[b] = True
                    start_batch(b)

            sp_of_job = [None] * n_jobs

            def s_mm(g):
                b, kt = jobs[g]
                ensure_started(b)
                # prefetch inputs for every batch reached within 6 jobs
                tb = jobs[min(g + 6, n_jobs - 1)][0]
                for bb in range(b + 1, tb + 1):
                    ensure_started(bb)
                sp = sp_pool.tile([128, L], F32, tag="sp", name=f"sp{g}")
                pt = pt_pool.tile([128, L], F16, tag="pt", name=f"pt{g}")
                sp_of_job[g] = sp
                pt_of_job[g] = pt
                ktile = kop_s[b][:, kt]   # [128, 2, 128]
                if g == 0:
                    # 256-col chunks: the first chunk only needs the qop lead
                    # DMA, so the first exp starts as early as possible
                    for qo in range(0, L, 256):
                        nc.tensor.matmul(
                            sp[:, qo:qo + 256], ktile,
                            qop_s[b][:, :, qo:qo + 256],
                            start=True, stop=True,
                            perf_mode=mybir.MatmulPerfMode.DoubleRow)
                else:
                    for qo in range(0, L, 512):
                        nc.tensor.matmul(
                            sp[:, qo:qo + 512], ktile,
                            qop_s[b][:, :, qo:qo + 512],
                            start=True, stop=True,
                            perf_mode=mybir.MatmulPerfMode.DoubleRow)

            def exp_emit(g):
                sp = sp_of_job[g]
                pt = pt_of_job[g]
                sp_of_job[g] = None
                if act_tile[g]:
                    nc.scalar.activation(pt[:], sp[:],
                                         mybir.ActivationFunctionType.Exp)
                else:
                    nc.vector.tensor_scalar(
                        pt.bitcast(I16)[:], sp[:], EXP_A, EXP_B,
                        mybir.AluOpType.mult, mybir.AluOpType.add)

            def o_mm(g):
                b, kt = jobs[g]
                nkt = ns[b]
                if kt == 0:
                    opsum[b] = [
                        op_pool.tile([128, 512], F32, tag=f"oph{h}",
                                     name=f"op{b}h{h}")
                        for h in range(2)
                    ]
                pt = pt_of_job[g]
                last = kt == nkt - 1
                if kt == 0 and (b == 0 or b == BPC - 1):
                    # open each bank with a zero-padded matmul: slot 0 opens
                    # undefined PSUM; the last slot opts out of the stale-
                    # accumulate chain so its output ships exact fp16
                    vz = vp_s[b][:, :260]  # [V|1|zeros(195)]
                    for h in range(2):
                        nc.tensor.matmul(
                            opsum[b][h][:, :260],
                            pt[:, h * 512:h * 512 + 128], vz,
                            start=True, stop=False,
                            skip_group_check=True)
                    qlist = [1, 2, 3, 5, 6, 7]
                else:
                    qlist = list(range(8))
                vt = vp_s[b][:, kt * (D + 1) + 195:kt * (D + 1) + 260] \
                    if kt > 0 else vp_s[b][:, :D + 1]
                for q in qlist:
                    h, j = divmod(q, 4)
                    nc.tensor.matmul(
                        opsum[b][h][:, j * (D + 1):(j + 1) * (D + 1)],
                        pt[:, q * 128:(q + 1) * 128],
                        vt,
                        start=False,
                        stop=last and (j == 3),
                        skip_group_check=True,
                    )
                pt_of_job[g] = None
                if last and b == BPC - 1:
                    # tail: per-bank copies in parallel (ACT || DVE), fp16,
                    # each bank DMA'd as soon as its copy lands (SP queue is
                    # idle at the end; HWDGE latency beats Pool SWDGE)
                    osbL = osb_pool.tile([128, 2, 4 * (D + 1)], F16,
                                         tag="osbL", name="osbL")
                    nc.scalar.copy(osbL[:, 0, :], opsum[b][0][:, :4 * (D + 1)])
                    nc.sync.dma_start(o16_d[:, 0], osbL[:, 0, :])
                    nc.vector.tensor_copy(osbL[:, 1, :],
                                          opsum[b][1][:, :4 * (D + 1)])
                    nc.sync.dma_start(o16_d[:, 1], osbL[:, 1, :])
                elif last:
                    osb = osb_pool.tile([128, 2, 4 * (D + 1)], F32,
                                        tag="osb", name=f"osb{b}")
                    nc.scalar.copy(osb[:, 0, :], opsum[b][0][:, :4 * (D + 1)])
                    nc.vector.tensor_copy(osb[:, 1, :],
                                          opsum[b][1][:, :4 * (D + 1)])
                    # Pool SWDGE: keeps the in-order SP queue free for input
                    # prefetch (a blocked output DMA would stall it)
                    nc.gpsimd.dma_start(o_d[b], osb[:])

            LOOKAHEAD = 5
            for g in range(n_jobs):
                s_mm(g)
                if g >= LOOKAHEAD:
                    o_mm(g - LOOKAHEAD)
                exp_emit(g)
            for g in range(max(0, n_jobs - LOOKAHEAD), n_jobs):
                o_mm(g)

    nc.compile()
    return nc


def get_program(ns):
    ns = tuple(ns)
    if ns not in _prog_cache:
        _prog_cache[ns] = _build_program(ns)
    return _prog_cache[ns]


def _prep_inputs(q, k, v, vl):
    """q,k,v: [n, L, D] fp32; vl: [n] int (vl>0).
    Returns (qop fp8 [n,128,2,L], kop fp8 [n,128,KT,2,128],
             vp fp16 [n,128,VPW])."""
    n = q.shape[0]
    s = np.float32(1.0 / np.sqrt(8.0))
    qt = (q.transpose(0, 2, 1) * s).astype(np.float32)   # [n, D, L]
    kt_ = (k.transpose(0, 2, 1) * s).astype(np.float32)  # [n, D, L]
    iota = np.arange(L)
    kmask = (iota[None, :] < vl[:, None])                # [n, L] valid keys
    kt_ = kt_ * kmask[:, None, :]

    qhi8 = qt.astype(E4)
    qlo8 = (qt - qhi8.astype(np.float32)).astype(E4)
    khi8 = kt_.astype(E4)
    klo8 = (kt_ - khi8.astype(np.float32)).astype(E4)

    # 4-term pack: partitions p<64 -> Qhi[d], p>=64 -> Qlo[d]; dim1 r is the
    # DoubleRow pair index; K side: r=0 -> Khi, r=1 -> Klo for p<64 and
    # r=0 -> Khi, r=1 -> Klo for p>=64 as well, so that the (p, r) terms are
    # (QhiKhi, QhiKlo, QloKhi, QloKlo).
    qop = np.empty((n, 128, 2, L), E4)
    qop[:, :D, 0] = qhi8
    qop[:, :D, 1] = qhi8
    qop[:, D:, 0] = qlo8
    qop[:, D:, 1] = qlo8

    kop = np.empty((n, 128, KT, 2, 128), E4)
    kk_hi = khi8.reshape(n, D, KT, 128)
    kk_lo = klo8.reshape(n, D, KT, 128)
    kop[:, :D, :, 0] = kk_hi
    kop[:, :D, :, 1] = kk_lo
    kop[:, D:, :, 0] = kk_hi
    kop[:, D:, :, 1] = kk_lo

    vpe = np.empty((n, L, D + 1), np.float16)
    vpe[:, :, :D] = v.astype(np.float16)
    vpe[:, :, D] = 1.0
    vpe *= kmask[:, :, None]     # masked keys contribute 0 to num and den
    vpe = vpe.reshape(n, KT, 128, D + 1).transpose(0, 2, 1, 3)
    vp = np.zeros((n, 128, VPW), np.float16)
    vp[:, :, :D + 1] = vpe[:, :, 0, :]
    vp[:, :, 260:] = vpe[:, :, 1:, :].reshape(n, 128, (KT - 1) * (D + 1))
    return qop, kop, vp


def kernel(queries, keys, values, valid_lens):
    queries = np.asarray(queries, np.float32)
    keys = np.asarray(keys, np.float32)
    values = np.asarray(values, np.float32)
    vl = np.asarray(valid_lens).astype(np.int64)

    # vl==0 -> reference softmaxes constant NEG_INF -> uniform over ALL keys
    zmask = vl == 0
    vl_dev = np.where(zmask, L, vl)

    # Ragged load balancing: sort batches by active k-tile count, deal
    # across cores (slot s <- sorted group). Slot order big->small: a big
    # first batch covers the early input-DMA ramp; the smallest last batch
    # leaves the shortest drain chain.
    nact = (-(-vl_dev // 128)).astype(np.int64)
    order = np.argsort(nact, kind="stable")
    # groups sorted ascending by size; slot order interleaves big and small
    # so snapshot copies spread through the job stream, with the biggest
    # first (covers the input-DMA ramp) and the smallest last (short drain)
    slot_groups = [7, 2, 5, 3, 6, 4, 1, 0]
    ns = tuple(int(nact[order[g * N_CORES + N_CORES - 1]])
               for g in slot_groups)

    qop, kop, vp = _prep_inputs(queries[order], keys[order], values[order],
                                vl_dev[order])

    nc = get_program(ns)
    in_maps = []
    for c in range(N_CORES):
        idx = [slot_groups[s] * N_CORES + c for s in range(BPC)]
        in_maps.append({
            "qop": np.ascontiguousarray(qop[idx]),
            "kop": np.ascontiguousarray(kop[idx]),
            "vp": np.ascontiguousarray(vp[idx]),
        })

    res = None
    for attempt in range(3):
        try:
            res = run_bass_kernel_spmd(nc, in_maps, list(range(N_CORES)))
            break
        except Exception:
            # Transient NRT/axon device failures have been observed on the
            # first execution of a freshly compiled NEFF; reset and retry.
            if attempt == 2:
                raise
            import time as _time
            _time.sleep(2.0)
            try:
                import jax
                jax.clear_caches()
            except Exception:
                pass

    out = np.empty((B, L, D), np.float32)
    for c in range(N_CORES):
        raw = res.results[c]["o"]  # [BPC-1,128,2,260] cumulative
        o = np.concatenate([
            raw[:1], np.diff(raw, axis=0),
            res.results[c]["o16"][None].astype(np.float32),
        ])
        o = o.reshape(BPC, 128, 2, 4, D + 1).transpose(0, 2, 3, 1, 4)
        o = o.reshape(BPC, L, D + 1)  # rows [(4h+j)*128 + p]
        on = o[:, :, :D] / o[:, :, D:D + 1]
        for s in range(BPC):
            out[order[slot_groups[s] * N_CORES + c]] = on[s]

    if zmask.any():
        out[zmask] = values[zmask].mean(axis=1, keepdims=True)
    return out
